# revision 26
# baseline (speedup 1.0000x reference)
"""MoE (63 routed experts, top-7, 1 shared expert) Trainium2 Bass kernel.

Strategy (expert parallelism, per sharding hint):
  - Host: router matmul + softmax + top-k (tiny: 0.7 GFLOP vs 220 GFLOP of
    expert FFNs), token gather per expert.
  - Device (8 NeuronCores, SPMD): each core runs 9 "units": 8 routed-expert
    slots (64 slots globally = 63 experts + 1 overflow slot) and 1
    shared-expert slot over a 1/8 token slice.
    Routed units run in fp8e4m3 with DoubleRow matmuls (2 fp8 weights per PE
    cell, K=256 per instruction): h = gelu((XeT^T @ (256*W1))/256 + b1);
    y*256 = h @ (256*W2).  Weights are pre-scaled by 256 (a power of two, so
    exact) to keep them in e4m3's normal range; the 1/256 is folded into the
    GELU's input scale on layer 1 and into the host-side gate scaling on
    layer 2.  The shared-expert unit (gate 1.0, so it dominates the error
    budget) runs in plain fp16 and is scheduled FIRST: it is DMA-light and
    compute-heavy, which lets the fp8 units' weight streams run ahead.
    All of a unit's remaining DMAs plus the next unit's input DMAs are
    issued at the top of each unit (software pipeline) so the PE never
    stalls at unit boundaries.  A short burst of dummy matmuls at t=0 warms
    the PE HAM clock gate (1.2 -> 2.4 GHz) before the first real matmul.
  - Host: scatter-add gated expert outputs (+ gate*b2), add shared out,
    bias and residual.

Experts are assigned to slots by descending load rank with static per-unit
token capacities (CAPS); both matmul layers' free dim is the capacity, so
PE cost tracks actual expert load.  Overload spills into the spare 64th
slot and, beyond that, to an exact host-side FFN for the few excess
tokens.  Gating and b2 are applied on the host during the scatter.
"""

import numpy as np

B, S, HID = 2, 2048, 1280
E = 63
I = 1280
TOP_K = 7
NCORES = 8
UNITS = 9          # 8 routed-expert slots + 1 shared-expert slot
RUNITS = 8         # routed units per core
C = 512            # token capacity per expert slot
KO = HID // 128    # 10 contraction chunks of 128
KP = KO // 2       # 5 DoubleRow contraction pairs (K=256 each)
T = B * S          # 4096
TSH = T // NCORES  # 512 shared-expert tokens per core

W1CW = 256          # w1 chunk width along I (2 lhsT column groups)
W2CW = 256          # w2 chunk width along H (2 lhsT column groups)
N_W1C = I // W1CW   # 5
N_W2C = HID // W2CW  # 5

WSCALE = 256.0      # power-of-two pre-scale for fp8 routed weights
WARM_MMS = 9        # dummy matmuls bridging DMA latency at kernel start

# Per-unit-index token capacities (unit 8 = shared). Experts are assigned
# to slots by load rank (rank r -> core r%8, unit r//8), so unit j only
# ever sees the j-th bucket of the descending load distribution; caps hug
# the bucket maxima of near-uniform routing. Uncovered overflow goes to
# the spare slot 63 and, beyond that, to an exact host fallback.
CAPS = [512, 492, 476, 464, 452, 440, 432, 424, C]

_cache = {}


def _build_nc():
    import concourse.mybir as mybir
    import concourse.tile as tile
    from concourse import bacc

    f32 = mybir.dt.float32
    f16 = mybir.dt.float16
    f8 = mybir.dt.float8e4
    GELU = mybir.ActivationFunctionType.Gelu
    DR = mybir.MatmulPerfMode.DoubleRow

    nc = bacc.Bacc(None, target_bir_lowering=False)

    # routed (fp8) inputs
    xg_d = nc.dram_tensor("xg", [RUNITS, 128, KO, C], f8, kind="ExternalInput")
    w1_d = nc.dram_tensor("w1", [RUNITS, N_W1C, 128, KO, W1CW], f8,
                          kind="ExternalInput")
    w2_d = nc.dram_tensor("w2", [RUNITS, N_W2C, 128, KO, W2CW], f8,
                          kind="ExternalInput")
    # shared inputs (mm1 in fp16; its mm2 runs fp8 DoubleRow like the rest,
    # so w2s is fp8 and the shared output is 256*y like the routed units)
    xs_d = nc.dram_tensor("xs", [128, KO, TSH], f16, kind="ExternalInput")
    w1s_d = nc.dram_tensor("w1s", [N_W1C, 128, KO, W1CW], f16,
                           kind="ExternalInput")
    w2s_d = nc.dram_tensor("w2s", [N_W2C, 128, KO, W2CW], f8,
                           kind="ExternalInput")
    b1_d = nc.dram_tensor("b1", [UNITS, 128, KO], f32, kind="ExternalInput")
    # transposed output: out[u, p, hk, c] = yscaled[token c, h = hk*128+p]
    # routed units hold 256*y (host folds 1/256 into gates); shared holds y.
    out_d = nc.dram_tensor("out", [UNITS, 128, KO, C], f16, kind="ExternalOutput")

    # Unit order: a small fp8 routed unit first (its input DMAs are half the
    # bytes of the fp16 shared unit's, so the PE starts sooner), then the
    # DMA-light compute-heavy shared unit (lets every other unit's weight
    # stream run ahead), then the rest, ending on the small spare slot.
    order = [6, 8, 0, 1, 2, 3, 4, 5, 7]

    def w1_src(u, ic):
        return w1s_d[ic] if u == 8 else w1_d[u, ic]

    def w2_src(u, ic):
        return w2s_d[ic] if u == 8 else w2_d[u, ic]

    with tile.TileContext(nc) as tc:
        with tc.tile_pool(name="xg_p", bufs=3) as xg_p, \
             tc.tile_pool(name="h1_p", bufs=2) as h1_p, \
             tc.tile_pool(name="w1_p", bufs=10) as w1_p, \
             tc.tile_pool(name="w2_p", bufs=8) as w2_p, \
             tc.tile_pool(name="out_p", bufs=2) as out_p, \
             tc.tile_pool(name="sm_p", bufs=2) as sm_p, \
             tc.tile_pool(name="wm_p", bufs=1) as wm_p, \
             tc.tile_pool(name="ps1_p", bufs=4, space="PSUM") as ps1_p, \
             tc.tile_pool(name="ps2_p", bufs=4, space="PSUM") as ps2_p:

            # ---- PE warm-up: keep the PE busy (and the HAM clock gate
            # opening) while the first unit's input DMAs land
            wz = wm_p.tile([128, C], f16, tag="wz")
            nc.any.memset(wz[:], 0)
            psw = ps1_p.tile([128, C], f32, tag="ps1", name="psw")
            for _ in range(WARM_MMS):
                nc.tensor.matmul(psw[:], wz[:, :128], wz[:], start=True,
                                 stop=True)

            # per-unit tiles created by the prefetch pipeline
            st = {u: {} for u in order}

            def issue_front(u, first=False):
                """xu halves + b1 + w1 chunks 0-2 for unit u."""
                mdt = f16 if u == 8 else f8
                CAP = CAPS[u]
                d = st[u]
                d["xu"] = xu = xg_p.tile([128, KO, C], mdt, tag="xu", name="xu")

                def xsrc(a, b):
                    return xs_d[:, a:b, :CAP] if u == 8 else xg_d[u, :, a:b, :CAP]

                d["w1"] = {}

                def w1tile():
                    return w1_p.tile([128, KO, W1CW], mdt, tag="w1c", name="w1c")

                if first:
                    # first unit: spread the issue across all three DMA
                    # initiator engines so the serial ~640ns descriptor-gen
                    # times run in parallel and the data lands sooner
                    nc.sync.dma_start(xu[:, :, :CAP], xsrc(0, KO))
                    w1c = d["w1"][0] = w1tile()
                    nc.scalar.dma_start(w1c[:], w1_src(u, 0))
                    nc.gpsimd.dma_start(b1u_early := sm_p.tile(
                        [128, KO], f32, tag="b1u", name="b1u"), b1_d[u])
                    d["b1"] = b1u_early
                    for ic in (1, 2):
                        w1c = d["w1"][ic] = w1tile()
                        nc.scalar.dma_start(w1c[:], w1_src(u, ic))
                    return
                else:
                    nc.sync.dma_start(xu[:, :KO // 2, :CAP], xsrc(0, KO // 2))
                    nc.sync.dma_start(xu[:, KO // 2:, :CAP], xsrc(KO // 2, KO))
                    w1c = d["w1"][0] = w1tile()
                    nc.sync.dma_start(w1c[:], w1_src(u, 0))
                d["b1"] = b1u = sm_p.tile([128, KO], f32, tag="b1u", name="b1u")
                nc.sync.dma_start(b1u[:], b1_d[u])
                for ic in (1, 2):
                    w1c = d["w1"][ic] = w1tile()
                    nc.sync.dma_start(w1c[:], w1_src(u, ic))

            issue_front(order[0], first=True)

            for pi, u in enumerate(order):
                CAP = CAPS[u]
                shared = (u == 8)
                mdt = f16 if shared else f8
                d = st[u]
                nxt = order[pi + 1] if pi + 1 < len(order) else None

                # ---- top-of-unit DMA issue (software pipeline) ----
                # w2 chunks are issued from the Scalar engine and outputs
                # from GpSimd so the Sync engine's serial DMA-issue stream
                # (with its head-of-line buffer-reuse waits) only carries
                # the mm1-critical tensors.
                # current unit's remaining w1 chunks (needed mid-mm1)
                for ic in (3, 4):
                    w1c = w1_p.tile([128, KO, W1CW], mdt, tag="w1c", name="w1c")
                    nc.sync.dma_start(w1c[:], w1_src(u, ic))
                    d["w1"][ic] = w1c
                # current unit's first w2 chunks (needed at mm2 start)
                d["w2"] = {}
                for ic in range(2):
                    w2c = w2_p.tile([128, KO, W2CW], f8, tag="w2c", name="w2c")
                    nc.scalar.dma_start(w2c[:], w2_src(u, ic))
                    d["w2"][ic] = w2c
                # next unit's inputs (needed at next unit start)
                if nxt is not None:
                    issue_front(nxt)

                xu = d["xu"]
                b1u = d["b1"]
                # h1 is fp8 for every unit: mm2 is DoubleRow fp8 everywhere
                h1 = h1_p.tile([128, KO, C], f8, tag="h1")

                # ---- mm1: h1[i, c] = gelu(sum_h W1[h,i] * X^T[h,c] + b1[i])
                for ic in range(N_W1C):
                    if ic == 1:
                        # remaining w2 chunks, after the first GELUs so the
                        # Scalar engine doesn't delay ps1 recycling
                        for jc in (2, 3, 4):
                            w2c = w2_p.tile([128, KO, W2CW], f8, tag="w2c",
                                            name="w2c")
                            nc.scalar.dma_start(w2c[:], w2_src(u, jc))
                            d["w2"][jc] = w2c
                    w1c = d["w1"][ic]
                    for s in range(W1CW // 128):
                        i_out = ic * (W1CW // 128) + s
                        ps = ps1_p.tile([128, C], f32, tag="ps1")
                        if shared:
                            for ko in range(KO):
                                nc.tensor.matmul(
                                    ps[:, :CAP],
                                    w1c[:, ko, s * 128:(s + 1) * 128],
                                    xu[:, ko, :CAP],
                                    start=(ko == 0),
                                    stop=(ko == KO - 1),
                                )
                        else:
                            for j in range(KP):
                                nc.tensor.matmul(
                                    ps[:, :CAP],
                                    w1c[:, 2 * j:2 * j + 2, s * 128:(s + 1) * 128],
                                    xu[:, 2 * j:2 * j + 2, :CAP],
                                    start=(j == 0),
                                    stop=(j == KP - 1),
                                    perf_mode=DR,
                                )
                        nc.scalar.activation(
                            h1[:, i_out, :CAP], ps[:, :CAP], GELU,
                            bias=b1u[:, i_out:i_out + 1],
                            scale=1.0 if shared else 1.0 / WSCALE)

                # ---- mm2 (transposed): yT[h, c] = sum_i W2[i, h] * h1[i, c]
                # gating and b2 are applied on the host during scatter.
                oy = out_p.tile([128, KO, C], f16, tag="oy")
                for hcc in range(N_W2C):
                    w2c = d["w2"][hcc]
                    for s2 in range(W2CW // 128):
                        hk = hcc * (W2CW // 128) + s2
                        ps2 = ps2_p.tile([128, C], f32, tag="ps2")
                        for j in range(KP):
                            nc.tensor.matmul(
                                ps2[:, :CAP],
                                w2c[:, 2 * j:2 * j + 2, s2 * 128:(s2 + 1) * 128],
                                h1[:, 2 * j:2 * j + 2, :CAP],
                                start=(j == 0),
                                stop=(j == KP - 1),
                                perf_mode=DR,
                            )
                        nc.vector.tensor_copy(oy[:, hk, :CAP], ps2[:, :CAP])
                        # drain finished output rows early so the final DMA
                        # (and the kernel tail) stays small.  The last unit
                        # drains per-row via the Scalar HWDGE (faster issue
                        # and completion than GpSimd's software DGE) so the
                        # kernel tail isn't gated on a slow final DMA.
                        last_unit = (pi == len(order) - 1)
                        if last_unit and hk >= 8:
                            nc.scalar.dma_start(out_d[u, :, hk:hk + 1, :CAP],
                                                oy[:, hk:hk + 1, :CAP])
                        elif hk % 2 == 1:
                            (nc.scalar if last_unit else nc.gpsimd).dma_start(
                                out_d[u, :, hk - 1:hk + 1, :CAP],
                                oy[:, hk - 1:hk + 1, :CAP])

    nc.compile()
    return nc


def _get_nc():
    if "nc" not in _cache:
        _cache["nc"] = _build_nc()
    return _cache["nc"]


def _f8():
    import ml_dtypes
    return np.dtype(ml_dtypes.float8_e4m3)


def _gelu_np(v):
    from scipy.special import erf
    v = v.astype(np.float32)
    return (0.5 * v * (1.0 + erf(v / np.sqrt(2.0)))).astype(np.float32)


def _tile_w1(w):
    # [H, I] -> [N_W1C, 128, KO, W1CW] with w1t[ic, p, ko, j] = w[ko*128+p, ic*W1CW+j]
    return w.reshape(KO, 128, N_W1C, W1CW).transpose(2, 1, 0, 3)


def _tile_w2(w):
    # [I, H] -> [N_W2C, 128, KO, W2CW]
    return w.reshape(KO, 128, N_W2C, W2CW).transpose(2, 1, 0, 3)


def _ensure_axon_hooks_stub():
    """bass_utils' axon trace path imports antenv.axon_hooks, which this
    image lacks; provide a no-op stub so a BASS_TRACE-enabled environment
    degrades gracefully instead of crashing."""
    import sys
    import types
    try:
        import antenv.axon_hooks  # noqa: F401
        return
    except ImportError:
        pass
    try:
        import antenv
    except ImportError:
        return
    mod = types.ModuleType("antenv.axon_hooks")
    holder = [None]
    mod.set_axon_ntff_profile_hook = lambda h: holder.__setitem__(0, h)
    mod.get_axon_ntff_profile_hook = lambda: holder[0]
    sys.modules["antenv.axon_hooks"] = mod
    antenv.axon_hooks = mod


def kernel(x, w1_shared, b1_shared, w2_shared, b2_shared,
           router_w, router_b, w1, b1, w2, b2):
    _ensure_axon_hooks_stub()
    from concourse.bass_utils import run_bass_kernel_spmd

    f8 = _f8()

    x = np.asarray(x, np.float32)
    w1 = np.asarray(w1, np.float32)
    b1 = np.asarray(b1, np.float32)
    w2 = np.asarray(w2, np.float32)
    b2 = np.asarray(b2, np.float32)
    w1_shared = np.asarray(w1_shared, np.float32)
    b1_shared = np.asarray(b1_shared, np.float32)
    w2_shared = np.asarray(w2_shared, np.float32)
    b2_shared = np.asarray(b2_shared, np.float32)
    router_w = np.asarray(router_w, np.float32)
    router_b = np.asarray(router_b, np.float32)

    xf = x.reshape(T, HID)

    # ---------------- host routing ----------------
    logits = xf @ router_w + router_b
    m = logits.max(-1, keepdims=True)
    ex = np.exp(logits - m, dtype=np.float32)
    affin = ex / ex.sum(-1, keepdims=True, dtype=np.float32)
    order = np.argsort(-affin, axis=-1, kind="stable")[:, :TOP_K]   # [T, K]
    vals = np.take_along_axis(affin, order, axis=-1)                # [T, K]

    # group (token, gate) pairs by expert
    flat_e = order.ravel()
    flat_t = np.repeat(np.arange(T), TOP_K)
    flat_g = vals.ravel()
    sort = np.argsort(flat_e, kind="stable")
    se, st, sg = flat_e[sort], flat_t[sort], flat_g[sort]
    starts = np.searchsorted(se, np.arange(E + 1))
    tok_by_e = [st[starts[e]:starts[e + 1]] for e in range(E)]
    gate_by_e = [sg[starts[e]:starts[e + 1]] for e in range(E)]

    # slot table: 64 expert slots; slot s = core*8 + unit.  Experts are
    # assigned by descending load rank: rank r -> core r%8, unit r//8, so
    # every core gets one expert from each load bucket and unit j's static
    # capacity CAPS[j] covers its bucket maximum.
    NSLOT = NCORES * 8
    slot_expert = [-1] * NSLOT
    slot_tok = [np.empty(0, np.int64)] * NSLOT
    slot_gate = [np.empty(0, np.float32)] * NSLOT
    ranked = sorted(range(E), key=lambda e: -len(tok_by_e[e]))
    overflow = []   # (expert, tokens, gates) beyond the primary slot cap
    for r, e in enumerate(ranked):
        s = (r % NCORES) * 8 + (r // NCORES)
        cap = CAPS[r // NCORES]
        slot_expert[s] = e
        slot_tok[s] = tok_by_e[e][:cap]
        slot_gate[s] = gate_by_e[e][:cap]
        if len(tok_by_e[e]) > cap:
            overflow.append((e, tok_by_e[e][cap:], gate_by_e[e][cap:]))
    # worst overflow spills into the spare slot 63 (unit 7, cap CAPS[7]);
    # anything further goes to an exact host fallback (rare).
    host_fallback = []
    if overflow:
        overflow.sort(key=lambda t: -len(t[1]))
        e0, t0, g0 = overflow[0]
        cap63 = CAPS[7]
        slot_expert[63] = e0
        slot_tok[63] = t0[:cap63]
        slot_gate[63] = g0[:cap63]
        if len(t0) > cap63:
            host_fallback.append((e0, t0[cap63:], g0[cap63:]))
        for e, t, g in overflow[1:]:
            host_fallback.append((e, t, g))

    # ---------------- build per-core device inputs ----------------
    # x transposed + partition-tiled: xT_t[ko, p, t] = x[t, ko*128+p]
    xT = np.ascontiguousarray(xf.T)
    xT_t8 = xT.astype(f8).reshape(KO, 128, T)
    xT_t16 = xT.astype(np.float16).reshape(KO, 128, T)

    w1t_sh = _tile_w1(w1_shared[0]).astype(np.float16)
    w2t_sh = _tile_w2(w2_shared[0] * WSCALE).astype(f8)
    b1t_sh = b1_shared[0].reshape(KO, 128).T

    # Rank-1 bias correction for the shared expert's fp8 mm2 (standard PTQ
    # bias correction): the w2 quantization error dW2 is identical for every
    # token, so its h-mean-weighted component is a fixed per-channel offset
    # c = mean_t(h8[t]) @ dW2 that we subtract on the host.  This removes
    # the token-correlated (systematic) part of the quantization error; the
    # remaining error is zero-mean across tokens.
    w2s_deq = (w2t_sh.astype(np.float32) / WSCALE) \
        .transpose(2, 1, 0, 3).reshape(I, HID)
    dW2 = w2s_deq - w2_shared[0]
    hh = _gelu_np(xf @ w1_shared[0] + b1_shared[0])
    h8 = hh.astype(f8).astype(np.float32)
    shared_bias_c = (h8.mean(0, dtype=np.float64) @ dW2).astype(np.float32)

    in_maps = []
    for c in range(NCORES):
        xg = np.zeros((RUNITS, 128, KO, C), f8)
        w1u = np.zeros((RUNITS, N_W1C, 128, KO, W1CW), f8)
        b1u = np.zeros((UNITS, 128, KO), np.float32)
        w2u = np.zeros((RUNITS, N_W2C, 128, KO, W2CW), f8)
        for u in range(RUNITS):
            s = c * 8 + u
            e = slot_expert[s]
            if e < 0 or len(slot_tok[s]) == 0:
                continue
            n = len(slot_tok[s])
            idx = np.zeros(C, np.int64)
            idx[:n] = slot_tok[s]
            xg[u] = xT_t8[:, :, idx].swapaxes(0, 1)
            w1u[u] = _tile_w1(w1[e] * WSCALE).astype(f8)
            b1u[u] = b1[e].reshape(KO, 128).T
            w2u[u] = _tile_w2(w2[e] * WSCALE).astype(f8)
        # shared-expert unit
        xs = xT_t16[:, :, c * TSH:(c + 1) * TSH].swapaxes(0, 1)
        b1u[8] = b1t_sh
        in_maps.append({"xg": xg, "w1": w1u, "b1": b1u, "w2": w2u,
                        "xs": np.ascontiguousarray(xs),
                        "w1s": w1t_sh, "w2s": w2t_sh})

    # ---------------- run on 8 cores ----------------
    nc = _get_nc()
    res = run_bass_kernel_spmd(nc, in_maps, core_ids=list(range(NCORES)))
    outs = [r["out"] for r in res.results]   # [UNITS, 128, KO, C] each

    # ---------------- host unshard / scatter ----------------
    # device output is transposed: outs[c][u][p, hk, c'] = ysc[c', hk*128+p]
    def untile_y(o, n):
        return o.transpose(1, 0, 2).reshape(HID, C)[:, :n].T.astype(np.float32)

    acc = np.zeros((T, HID), np.float32)     # shared + routed
    inv = np.float32(1.0 / WSCALE)
    # shared expert (unit 8 on each core), gate 1, + b2_shared; the device
    # holds 256*y (fp8 mm2 with pre-scaled weights)
    for c in range(NCORES):
        ys = untile_y(outs[c][8], TSH)
        acc[c * TSH:(c + 1) * TSH] = ys * inv + (b2_shared[0] - shared_bias_c)
    # routed experts: gate * (y + b2), scattered by token; device holds
    # 256*y so fold the 1/256 into the gate.
    for s in range(NCORES * 8):
        e = slot_expert[s]
        n = len(slot_tok[s])
        if e < 0 or n == 0:
            continue
        ye = untile_y(outs[s // 8][s % 8], n)
        # token indices are unique within one slot, so fancy += is safe
        acc[slot_tok[s]] += (slot_gate[s] * inv)[:, None] * ye \
            + slot_gate[s][:, None] * b2[e][None, :]
    # exact host fallback for overflow beyond device capacity
    for e, toks, gs in host_fallback:
        h = _gelu_np(xf[toks] @ w1[e] + b1[e])
        acc[toks] += gs[:, None] * (h @ w2[e] + b2[e])

    return (acc + xf).reshape(B, S, HID).astype(np.float32)
